# revision 16
# baseline (speedup 1.0000x reference)
"""Trainium2 Bass kernel for AudioAdapterAttnProcessor.

Reference computation (B=4, S=4096, D=1024, H=16, HD=64, C=768,
S_TXT=77, S_AUD=16):
    q = (hidden @ Wq)                                  [B, S, H, HD]
    base  = softmax(q k_t^T / 8) v_t   (text cross-attn, k/v from encoder)
    audio = softmax(q k_a^T / 8) v_a   (audio cross-attn)
    out = concat_heads(base + audio) @ Wo + bo

Sharding: (batch x seq-half) -> 8 cores; each core handles one batch's
2048 queries for all 16 heads.  No collectives: every query row of the
output depends only on its own hidden row (cross-attention to 93 fixed
keys per batch), so the gather is a pure concatenation.

Per-core device program (everything transposed so PE contracts naturally):
    qT   = Wq^T @ hiddenT                (hiddenT fed pre-transposed, bf16)
    s    = q_h^T.T @ kcatT_h             natural scores [128q, 93keys]
    p    = exp(s/8) (fused scale); denominators via segmented DVE reduces
    pn   = p * recip(den)                written into a [128q,(h,qt),128k]
                                         zero-padded staging tile
    pT   = DMA-xbar transpose of pn      ONE batched 3D transpose per chunk
                                         -> expT [128k,(h,qt),128q]; frees
                                         the PE of 64 transposes/chunk and
                                         the ACT engine of the PSUM copies
    pv   = v_h(pad128)^T.T @ expT_h      -> attn_outT [64, 512] per head
    out  = attn_T.T @ Wo                 natural [128q, 1024] fp32 -> DRAM
bo is added on the host during the gather (it is per-output-feature,
which is the free dim of the natural-layout output).

Software pipeline (PE stream order, steady state):
    qproj(c+1) | pv(c) | scores+norm(c+1) | outproj(c)
so the chunk-c DMA transpose (kicked right after norm(c)) is covered by
the 13.6us qproj(c+1), and the at-copy latency of pv(c) is covered by
the scores of c+1 before outproj(c) consumes at_t.
"""

import sys

sys.path.insert(0, "/opt/trn_rl_repo")

from contextlib import ExitStack

import numpy as np
import ml_dtypes

import concourse.bass as bass
import concourse.mybir as mybir
import concourse.tile as tile
from concourse import bacc

BF16 = ml_dtypes.bfloat16

B, S, D = 4, 4096, 1024
S_TXT, S_AUD = 77, 16
C = 768
H = 16
HD = 64
NK = S_TXT + S_AUD  # 93 keys after concat
P = 128
SCALE = 1.0 / np.sqrt(HD)  # 0.125

N_CORES = 8
SEQ_PER_CORE = S // 2  # 2048
CHUNK = 512
N_CHUNKS = SEQ_PER_CORE // CHUNK  # 4
KT = D // P  # 8 contraction tiles
QT_PER_CHUNK = CHUNK // P  # 4


DEFAULT_CFG = dict(hidden=3, qT=2, probs_u=4, dsum=2, attn_T=2,
                   out_sb=4, accps=4, sps=2, vps=2,
                   prio_qT=0, prio_at=0, qT_dve=True,
                   at_dve=False, out_act=True, tsplit=2,
                   pu_bf16=True, norm_pool=True,
                   phase1_split=True, at_alt=False, out_alt=False,
                   prio_out=0)


def build_bass(cfg=None, reps=1):
    """Build the SPMD single-core Bass program (same program on all 8 cores).

    reps > 1 repeats the whole computation back-to-back inside the NEFF
    (same inputs -> same outputs); used only for slope-based timing.
    """
    cfg = {**DEFAULT_CFG, **(cfg or {})}
    nc = bacc.Bacc("TRN2", target_bir_lowering=False, debug=False, num_devices=N_CORES)

    ht_d = nc.dram_tensor("ht", [P, KT * SEQ_PER_CORE], mybir.dt.bfloat16, kind="ExternalInput")
    wq_d = nc.dram_tensor("wq", [P, KT * D], mybir.dt.bfloat16, kind="ExternalInput")
    wo_d = nc.dram_tensor("wo", [P, KT * D], mybir.dt.bfloat16, kind="ExternalInput")
    kc_d = nc.dram_tensor("kc", [P, KT * NK], mybir.dt.bfloat16, kind="ExternalInput")
    v_d = nc.dram_tensor("v", [P, D], mybir.dt.bfloat16, kind="ExternalInput")
    out_d = nc.dram_tensor("out", [SEQ_PER_CORE, D], mybir.dt.float32, kind="ExternalOutput")

    ht_view = ht_d[:].rearrange("p (k q) -> p k q", k=KT)  # [128, 8, 2048]

    import contextlib

    with tile.TileContext(nc) as tc, ExitStack() as ctx:
        def gate_copy(dst, src, prio, on_dve):
            """PSUM->SBUF copy that gates PE work; optionally boosted/moved."""
            cm = (tc.high_priority(None if prio < 0 else prio)
                  if prio else contextlib.nullcontext())
            with cm:
                if on_dve:
                    nc.vector.tensor_copy(dst, src)
                else:
                    nc.scalar.copy(dst, src)

        wpool = ctx.enter_context(tc.tile_pool(name="weights", bufs=1))
        hpool = ctx.enter_context(tc.tile_pool(name="hidden", bufs=cfg["hidden"]))
        qpool = ctx.enter_context(tc.tile_pool(name="qT", bufs=cfg["qT"]))
        ppool = ctx.enter_context(tc.tile_pool(name="probs_u", bufs=cfg["probs_u"]))
        dpool = ctx.enter_context(tc.tile_pool(name="dsum", bufs=cfg["dsum"]))
        atpool = ctx.enter_context(tc.tile_pool(name="attn_T", bufs=cfg["attn_T"]))
        opool = ctx.enter_context(tc.tile_pool(name="out_sb", bufs=cfg["out_sb"]))

        accps = ctx.enter_context(tc.tile_pool(name="accps", bufs=cfg["accps"], space="PSUM"))
        sps = ctx.enter_context(tc.tile_pool(name="sps", bufs=cfg["sps"], space="PSUM"))
        vps = ctx.enter_context(tc.tile_pool(name="vps", bufs=cfg["vps"], space="PSUM"))

        # One-time loads
        wq_t = wpool.tile([P, KT * D], mybir.dt.bfloat16)
        wo_t = wpool.tile([P, KT * D], mybir.dt.bfloat16)
        kc_t = wpool.tile([P, KT * NK], mybir.dt.bfloat16)
        v_t = wpool.tile([P, D], mybir.dt.bfloat16)
        # DMA issue order matters: the HWDGE queue drains in order, so put
        # everything the first q-projection needs ahead of the 2MB wo load.
        # Per-k-tile splits let matmuls start on sub-tile deps.
        ht_t0 = hpool.tile([P, KT, CHUNK], mybir.dt.bfloat16, tag="ht")
        for kt in range(KT):
            nc.sync.dma_start(wq_t[:, kt * D : (kt + 1) * D], wq_d[:, kt * D : (kt + 1) * D])
            nc.sync.dma_start(ht_t0[:, kt, :], ht_view[:, kt, 0:CHUNK])
        nc.sync.dma_start(kc_t[:], kc_d[:])
        nc.sync.dma_start(v_t[:], v_d[:])
        for kt in range(KT):
            nc.sync.dma_start(wo_t[:, kt * D : (kt + 1) * D], wo_d[:, kt * D : (kt + 1) * D])

        # Double-buffered staging for normalized probs (natural layout,
        # [128q, head, qtile, 128keys]; key cols 93:128 stay zero from the
        # one-time memset so the transposed pad rows are finite) and for the
        # transposed probs (keys on partitions).
        pn_bufs = [wpool.tile([P, H, QT_PER_CHUNK, P], mybir.dt.bfloat16,
                              name=f"pn{i}") for i in range(2)]
        expT_bufs = [wpool.tile([P, H * QT_PER_CHUNK * P], mybir.dt.bfloat16,
                                name=f"expT{i}") for i in range(2)]
        for t in pn_bufs:
            nc.gpsimd.memset(t[:], 0.0)

        def emit_ht(c, tile_=None):
            ht_t = tile_ or hpool.tile([P, KT, CHUNK], mybir.dt.bfloat16, tag="ht")
            if tile_ is None:
                nc.sync.dma_start(ht_t[:], ht_view[:, :, c * CHUNK : (c + 1) * CHUNK])
            return ht_t

        def emit_qproj_mt(ht_t, qT_t, mt):
            qp = accps.tile([P, CHUNK], mybir.dt.float32, tag="acc")
            for kt in range(KT):
                nc.tensor.matmul(
                    qp[:],
                    lhsT=wq_t[:, kt * D + mt * P : kt * D + (mt + 1) * P],
                    rhs=ht_t[:, kt, :],
                    start=(kt == 0),
                    stop=(kt == KT - 1),
                )
            gate_copy(qT_t[:, mt, :], qp[:], cfg["prio_qT"], cfg["qT_dve"])

        def kick_transpose(pn_t, expT_t, lo, hi):
            """DMA-xbar transpose pn[:, lo:hi] -> expT[:, lo*4*128:...].

            Issued on the ACT HWDGE so it never head-of-line blocks the SP
            queue that streams ht / out.  One 3D instruction batch-transposes
            (hi-lo)*4 independent [128q, 128k] tiles.
            """
            n = (hi - lo) * QT_PER_CHUNK
            out_view = expT_t[:, lo * QT_PER_CHUNK * P : hi * QT_PER_CHUNK * P]
            out3 = out_view.rearrange("p (j q) -> p j q", j=n)
            nc.scalar.dma_start(out3, pn_t[:, lo:hi], transpose=True)

        pdt = mybir.dt.bfloat16 if cfg["pu_bf16"] else mybir.dt.float32
        norm_eng = nc.gpsimd if cfg["norm_pool"] else nc.vector

        def make_dsum():
            dsum = dpool.tile([P, H * 8], pdt)
            rds = dpool.tile([P, H * 8], pdt, tag="rds")
            return dsum, rds

        def emit_scores_pair(c, g, qT_t, pn_t, expT_t, dsum, rds):
            """scores -> exp -> segmented denominators -> normalized probs
            for head pair g (written into pn_t); kicks the transpose when a
            tsplit boundary is reached."""
            nsplit = cfg["tsplit"]
            for hh in range(2):
                h = 2 * g + hh
                off = hh * HD
                d0 = h * 8
                sp = sps.tile([P, QT_PER_CHUNK * NK], mybir.dt.float32)
                pu = ppool.tile([P, QT_PER_CHUNK * NK], pdt)
                for qt in range(QT_PER_CHUNK):
                    nc.tensor.matmul(
                        sp[:, qt * NK : (qt + 1) * NK],
                        lhsT=qT_t[off : off + HD, g, qt * P : (qt + 1) * P],
                        rhs=kc_t[off : off + HD, g * NK : (g + 1) * NK],
                        start=True,
                        stop=True,
                    )
                nc.scalar.activation(
                    pu[:],
                    sp[:],
                    mybir.ActivationFunctionType.Exp,
                    scale=float(SCALE),
                )
                pu3 = pu[:].rearrange("p (q k) -> p q k", q=QT_PER_CHUNK)
                with nc.allow_low_precision(reason="93-term softmax denominator; bf16 keeps the DVE reduce in 2x mode"):
                    nc.vector.reduce_sum(
                        dsum[:, d0 : d0 + 4], pu3[:, :, 0:S_TXT],
                        axis=mybir.AxisListType.X,
                    )
                    nc.vector.reduce_sum(
                        dsum[:, d0 + 4 : d0 + 8], pu3[:, :, S_TXT:NK],
                        axis=mybir.AxisListType.X,
                    )
                with nc.allow_low_precision(reason="bf16 reciprocal of softmax denominator"):
                    nc.vector.reciprocal(rds[:, d0 : d0 + 8], dsum[:, d0 : d0 + 8])
                # batched normalize into the staging tile: one op per
                # softmax, broadcasting the per-(partition, qtile)
                # reciprocal along keys (step-0 AP)
                pn3 = pn_t[:, h]  # [128, QT, 128], key cols 93: stay 0
                norm_eng.tensor_tensor(
                    pn3[:, :, 0:S_TXT],
                    pu3[:, :, 0:S_TXT],
                    rds[:, d0 : d0 + 4, None].to_broadcast([P, QT_PER_CHUNK, S_TXT]),
                    mybir.AluOpType.mult,
                )
                norm_eng.tensor_tensor(
                    pn3[:, :, S_TXT:NK],
                    pu3[:, :, S_TXT:NK],
                    rds[:, d0 + 4 : d0 + 8, None].to_broadcast([P, QT_PER_CHUNK, S_AUD]),
                    mybir.AluOpType.mult,
                )
            span = (H // 2) // nsplit
            if (g + 1) % span == 0:
                lo = (g + 1 - span) * 2
                kick_transpose(pn_t, expT_t, lo, (g + 1) * 2)

        def emit_pv_pair(g, expT_t, at_t):
            ev = expT_t[:].rearrange("p (h q) -> p h q", h=H)
            pvp = vps.tile([P, CHUNK], mybir.dt.float32)
            for hh in range(2):
                h = 2 * g + hh
                off = hh * HD
                nc.tensor.matmul(
                    pvp[off : off + HD, :],
                    lhsT=v_t[:, h * HD : (h + 1) * HD],
                    rhs=ev[:, h, :],
                    start=True,
                    stop=True,
                )
            # attn_T tile g holds heads 2g (rows 0-63) and 2g+1 (64-127)
            on_dve = (g % 2 == 1) if cfg["at_alt"] else cfg["at_dve"]
            gate_copy(at_t[:, g, :], pvp[:], cfg["prio_at"], on_dve)

        def emit_outproj_group(c, at_t, j):
            qt, nb = j // 2, j % 2
            op = accps.tile([P, CHUNK], mybir.dt.float32, tag="acc")
            for kt in range(KT):
                nc.tensor.matmul(
                    op[:],
                    lhsT=at_t[:, kt, qt * P : (qt + 1) * P],
                    rhs=wo_t[:, kt * D + nb * CHUNK : kt * D + (nb + 1) * CHUNK],
                    start=(kt == 0),
                    stop=(kt == KT - 1),
                )
            ob = opool.tile([P, CHUNK], mybir.dt.float32)
            on_act = (j % 2 == 0) if cfg["out_alt"] else cfg["out_act"]
            cm = (tc.high_priority(None if cfg["prio_out"] < 0 else cfg["prio_out"])
                  if cfg["prio_out"] else contextlib.nullcontext())
            with cm:
                if on_act:
                    nc.scalar.copy(ob[:], op[:])
                else:
                    nc.vector.tensor_copy(ob[:], op[:])
            nc.sync.dma_start(
                out_d[
                    c * CHUNK + qt * P : c * CHUNK + (qt + 1) * P,
                    nb * CHUNK : (nb + 1) * CHUNK,
                ],
                ob[:],
            )

        chunks = [c for _ in range(reps) for c in range(N_CHUNKS)]
        # prologue: load chunk 0, then qproj-mt g immediately followed by the
        # scores for head pair g (which only needs qT[:, g]) so the ACT/DVE
        # softmax chain overlaps the remaining projections.
        ht_t = emit_ht(chunks[0], ht_t0)
        if len(chunks) > 1:
            ht_next = emit_ht(chunks[1])
        qT_t = qpool.tile([P, KT, CHUNK], mybir.dt.bfloat16)
        dsum, rds = make_dsum()
        for g in range(KT):
            emit_qproj_mt(ht_t, qT_t, g)
            emit_scores_pair(chunks[0], g, qT_t, pn_bufs[0], expT_bufs[0],
                             dsum, rds)
        for i, c in enumerate(chunks):
            nxt = chunks[i + 1] if i + 1 < len(chunks) else None
            if i + 2 < len(chunks):
                ht_fut = emit_ht(chunks[i + 2])
            # phase 1: qproj(c+1) interleaved with pv(c)
            at_t = atpool.tile([P, KT, CHUNK], mybir.dt.bfloat16, tag="at")
            if nxt is not None:
                qT_next = qpool.tile([P, KT, CHUNK], mybir.dt.bfloat16)
            if cfg["phase1_split"]:
                if nxt is not None:
                    for g in range(KT):
                        emit_qproj_mt(ht_next, qT_next, g)
                for g in range(KT):
                    emit_pv_pair(g, expT_bufs[i % 2], at_t)
            else:
                for g in range(KT):
                    if nxt is not None:
                        emit_qproj_mt(ht_next, qT_next, g)
                    emit_pv_pair(g, expT_bufs[i % 2], at_t)
            # phase 2: scores/norm(c+1) interleaved with outproj(c)
            if nxt is not None:
                dsum, rds = make_dsum()
            for g in range(KT):
                if nxt is not None:
                    emit_scores_pair(nxt, g, qT_next, pn_bufs[(i + 1) % 2],
                                     expT_bufs[(i + 1) % 2], dsum, rds)
                emit_outproj_group(c, at_t, g)
            if nxt is not None:
                qT_t = qT_next
                ht_next = ht_fut if i + 2 < len(chunks) else None

    nc.compile()
    return nc


def _host_prep(hidden_states, encoder_hidden_states, audio_hidden_states,
               Wq, Wk, Wv, Wk_audio, Wv_audio, Wo):
    """Build the per-core input maps (all layouts pre-arranged on host)."""
    wq_sb = np.ascontiguousarray(
        Wq.reshape(KT, P, D).transpose(1, 0, 2).reshape(P, KT * D)
    ).astype(BF16)
    wo_sb = np.ascontiguousarray(
        Wo.reshape(KT, P, D).transpose(1, 0, 2).reshape(P, KT * D)
    ).astype(BF16)

    in_maps = []
    for b in range(B):
        # kv projections for this batch: tiny, done on host
        k_full = np.concatenate(
            [encoder_hidden_states[b] @ Wk, audio_hidden_states[b] @ Wk_audio], axis=0
        )  # [93, 1024]
        v_full = np.concatenate(
            [encoder_hidden_states[b] @ Wv, audio_hidden_states[b] @ Wv_audio], axis=0
        )  # [93, 1024]
        kc_sb = np.ascontiguousarray(
            k_full.T.reshape(KT, P, NK).transpose(1, 0, 2).reshape(P, KT * NK)
        ).astype(BF16)
        v_sb = np.zeros((P, D), dtype=BF16)
        v_sb[:NK] = v_full.astype(BF16)

        for half in range(2):
            rows = hidden_states[b, half * SEQ_PER_CORE : (half + 1) * SEQ_PER_CORE]
            ht_sb = np.ascontiguousarray(
                rows.T.reshape(KT, P, SEQ_PER_CORE)
                .transpose(1, 0, 2)
                .reshape(P, KT * SEQ_PER_CORE)
            ).astype(BF16)
            in_maps.append(
                {"ht": ht_sb, "wq": wq_sb, "wo": wo_sb, "kc": kc_sb, "v": v_sb}
            )
    return in_maps


_NC_CACHE = {}


def get_nc():
    if "nc" not in _NC_CACHE:
        _NC_CACHE["nc"] = build_bass()
    return _NC_CACHE["nc"]


def kernel(hidden_states, encoder_hidden_states, audio_hidden_states,
           Wq, Wk, Wv, Wk_audio, Wv_audio, Wo, bo):
    from concourse import bass_utils

    hidden_states = np.asarray(hidden_states, dtype=np.float32)
    encoder_hidden_states = np.asarray(encoder_hidden_states, dtype=np.float32)
    audio_hidden_states = np.asarray(audio_hidden_states, dtype=np.float32)
    Wq = np.asarray(Wq, dtype=np.float32)
    Wk = np.asarray(Wk, dtype=np.float32)
    Wv = np.asarray(Wv, dtype=np.float32)
    Wk_audio = np.asarray(Wk_audio, dtype=np.float32)
    Wv_audio = np.asarray(Wv_audio, dtype=np.float32)
    Wo = np.asarray(Wo, dtype=np.float32)
    bo = np.asarray(bo, dtype=np.float32)

    nc = get_nc()
    in_maps = _host_prep(hidden_states, encoder_hidden_states, audio_hidden_states,
                         Wq, Wk, Wv, Wk_audio, Wv_audio, Wo)
    res = bass_utils.run_bass_kernel_spmd(nc, in_maps, list(range(N_CORES)))

    out = np.empty((B, S, D), dtype=np.float32)
    core = 0
    for b in range(B):
        for half in range(2):
            out[b, half * SEQ_PER_CORE : (half + 1) * SEQ_PER_CORE] = res.results[core]["out"]
            core += 1
    out += bo[None, None, :]
    return out


# revision 27
# speedup vs baseline: 1.1060x; 1.1060x over previous
"""Trainium2 Bass kernel for AudioAdapterAttnProcessor.

Reference computation (B=4, S=4096, D=1024, H=16, HD=64, C=768,
S_TXT=77, S_AUD=16):
    q = (hidden @ Wq)                                  [B, S, H, HD]
    base  = softmax(q k_t^T / 8) v_t   (text cross-attn, k/v from encoder)
    audio = softmax(q k_a^T / 8) v_a   (audio cross-attn)
    out = concat_heads(base + audio) @ Wo + bo

Sharding: (batch x seq-half) -> 8 cores; each core handles one batch's
2048 queries for all 16 heads.  No collectives: every query row of the
output depends only on its own hidden row (cross-attention to 93 fixed
keys per batch), so the gather is a pure concatenation.

Per-core device program (everything transposed so PE contracts naturally):
    qT   = Wq^T @ hiddenT                (hiddenT fed pre-transposed, bf16)
    s    = q_h^T.T @ kcatT_h             natural scores [128q, 93keys]
    p    = exp(s/8) (fused scale); denominators via segmented DVE reduces
    pn   = p * recip(den)                written into a [128q,(h,qt),128k]
                                         zero-padded staging tile
    pT   = DMA-xbar transpose of pn      ONE batched 3D transpose per chunk
                                         -> expT [128k,(h,qt),128q]; frees
                                         the PE of 64 transposes/chunk and
                                         the ACT engine of the PSUM copies
    pv   = v_h(pad128)^T.T @ expT_h      -> attn_outT [64, 512] per head
    out  = attn_T.T @ Wo                 natural [128q, 1024] fp32 -> DRAM
bo is added on the host during the gather (it is per-output-feature,
which is the free dim of the natural-layout output).

Software pipeline (PE stream order, steady state):
    qproj(c+1) | pv(c) | scores+norm(c+1) | outproj(c)
so the chunk-c DMA transpose (kicked right after norm(c)) is covered by
the 13.6us qproj(c+1), and the at-copy latency of pv(c) is covered by
the scores of c+1 before outproj(c) consumes at_t.
"""

import sys

sys.path.insert(0, "/opt/trn_rl_repo")

from contextlib import ExitStack

import numpy as np
import ml_dtypes

import concourse.bass as bass
import concourse.mybir as mybir
import concourse.tile as tile
from concourse import bacc

BF16 = ml_dtypes.bfloat16

B, S, D = 4, 4096, 1024
S_TXT, S_AUD = 77, 16
C = 768
H = 16
HD = 64
NK = S_TXT + S_AUD  # 93 keys after concat
P = 128
SCALE = 1.0 / np.sqrt(HD)  # 0.125

N_CORES = 8
SEQ_PER_CORE = S // 2  # 2048
CHUNK = 512
N_CHUNKS = SEQ_PER_CORE // CHUNK  # 4
KT = D // P  # 8 contraction tiles
QT_PER_CHUNK = CHUNK // P  # 4


DEFAULT_CFG = dict(hidden=3, qT=2, probs_u=4, dsum=2, attn_T=2,
                   out_sb=4, accps=4, sps=2, vps=2,
                   prio_qT=0, prio_at=0, qT_dve=True,
                   at_dve=False, out_act=True, tsplit=2,
                   pu_bf16=False, norm_pool=False,
                   phase1_split=True, at_alt=False, out_alt=False,
                   prio_out=0, tsplit0=2, deep_prologue=False)


def build_bass(cfg=None, reps=1):
    """Build the SPMD single-core Bass program (same program on all 8 cores).

    reps > 1 repeats the whole computation back-to-back inside the NEFF
    (same inputs -> same outputs); used only for slope-based timing.
    """
    cfg = {**DEFAULT_CFG, **(cfg or {})}
    nc = bacc.Bacc("TRN2", target_bir_lowering=False, debug=False, num_devices=N_CORES)

    ht_d = nc.dram_tensor("ht", [P, KT * SEQ_PER_CORE], mybir.dt.bfloat16, kind="ExternalInput")
    wq_d = nc.dram_tensor("wq", [P, KT * D], mybir.dt.bfloat16, kind="ExternalInput")
    wo_d = nc.dram_tensor("wo", [P, KT * D], mybir.dt.bfloat16, kind="ExternalInput")
    kc_d = nc.dram_tensor("kc", [P, KT * NK], mybir.dt.bfloat16, kind="ExternalInput")
    v_d = nc.dram_tensor("v", [P, D], mybir.dt.bfloat16, kind="ExternalInput")
    out_d = nc.dram_tensor("out", [SEQ_PER_CORE, D], mybir.dt.float32, kind="ExternalOutput")

    ht_view = ht_d[:].rearrange("p (k q) -> p k q", k=KT)  # [128, 8, 2048]

    import contextlib

    with tile.TileContext(nc) as tc, ExitStack() as ctx:
        def gate_copy(dst, src, prio, on_dve):
            """PSUM->SBUF copy that gates PE work; optionally boosted/moved."""
            cm = (tc.high_priority(None if prio < 0 else prio)
                  if prio else contextlib.nullcontext())
            with cm:
                if on_dve:
                    nc.vector.tensor_copy(dst, src)
                else:
                    nc.scalar.copy(dst, src)

        wpool = ctx.enter_context(tc.tile_pool(name="weights", bufs=1))
        hpool = ctx.enter_context(tc.tile_pool(name="hidden", bufs=cfg["hidden"]))
        qpool = ctx.enter_context(tc.tile_pool(name="qT", bufs=cfg["qT"]))
        ppool = ctx.enter_context(tc.tile_pool(name="probs_u", bufs=cfg["probs_u"]))
        dpool = ctx.enter_context(tc.tile_pool(name="dsum", bufs=cfg["dsum"]))
        atpool = ctx.enter_context(tc.tile_pool(name="attn_T", bufs=cfg["attn_T"]))
        opool = ctx.enter_context(tc.tile_pool(name="out_sb", bufs=cfg["out_sb"]))

        accps = ctx.enter_context(tc.tile_pool(name="accps", bufs=cfg["accps"], space="PSUM"))
        sps = ctx.enter_context(tc.tile_pool(name="sps", bufs=cfg["sps"], space="PSUM"))
        vps = ctx.enter_context(tc.tile_pool(name="vps", bufs=cfg["vps"], space="PSUM"))

        # One-time loads
        wq_t = wpool.tile([P, KT * D], mybir.dt.bfloat16)
        wo_t = wpool.tile([P, KT * D], mybir.dt.bfloat16)
        kc_t = wpool.tile([P, KT * NK], mybir.dt.bfloat16)
        v_t = wpool.tile([P, D], mybir.dt.bfloat16)
        # DMA issue order matters: the HWDGE queue drains in order, so put
        # everything the first q-projection needs ahead of the 2MB wo load.
        # Per-k-tile splits let matmuls start on sub-tile deps.
        ht_t0 = hpool.tile([P, KT, CHUNK], mybir.dt.bfloat16, tag="ht")
        # wq is laid out mt-major on the host ([P, mt, kt, 128]) so qproj
        # group mt only waits on its own contiguous 256KB slice; ht0 per-kt
        # slices interleave ahead since group 0 reads every kt of ht.
        wq_dv = wq_d[:].rearrange("p (m c) -> p m c", m=KT)
        wq_tv = wq_t[:].rearrange("p (m c) -> p m c", m=KT)
        wq_mt = wq_t[:].rearrange("p (m k c) -> p m k c", m=KT, k=KT)
        for kt in range(KT):
            nc.sync.dma_start(ht_t0[:, kt, :], ht_view[:, kt, 0:CHUNK])
            if kt < 2:
                nc.sync.dma_start(wq_tv[:, kt], wq_dv[:, kt])
        nc.sync.dma_start(kc_t[:], kc_d[:])
        nc.sync.dma_start(v_t[:], v_d[:])
        for mt in range(2, KT):
            nc.sync.dma_start(wq_tv[:, mt], wq_dv[:, mt])
        for kt in range(KT):
            nc.sync.dma_start(wo_t[:, kt * D : (kt + 1) * D], wo_d[:, kt * D : (kt + 1) * D])

        # Double-buffered staging for normalized probs (natural layout,
        # [128q, head, qtile, 128keys]; key cols 93:128 stay zero from the
        # one-time memset so the transposed pad rows are finite) and for the
        # transposed probs (keys on partitions).
        pn_bufs = [wpool.tile([P, H, QT_PER_CHUNK, P], mybir.dt.bfloat16,
                              name=f"pn{i}") for i in range(2)]
        expT_bufs = [wpool.tile([P, H * QT_PER_CHUNK * P], mybir.dt.bfloat16,
                                name=f"expT{i}") for i in range(2)]
        for t in pn_bufs:
            nc.gpsimd.memset(t[:], 0.0)

        def emit_ht(c, tile_=None):
            ht_t = tile_ or hpool.tile([P, KT, CHUNK], mybir.dt.bfloat16, tag="ht")
            if tile_ is None:
                nc.sync.dma_start(ht_t[:], ht_view[:, :, c * CHUNK : (c + 1) * CHUNK])
            return ht_t

        def emit_qproj_mt(ht_t, qT_t, mt):
            qp = accps.tile([P, CHUNK], mybir.dt.float32, tag="acc")
            for kt in range(KT):
                nc.tensor.matmul(
                    qp[:],
                    lhsT=wq_mt[:, mt, kt, :],
                    rhs=ht_t[:, kt, :],
                    start=(kt == 0),
                    stop=(kt == KT - 1),
                )
            gate_copy(qT_t[:, mt, :], qp[:], cfg["prio_qT"], cfg["qT_dve"])

        def kick_transpose(pn_t, expT_t, lo, hi):
            """DMA-xbar transpose pn[:, lo:hi] -> expT[:, lo*4*128:...].

            Issued on the ACT HWDGE so it never head-of-line blocks the SP
            queue that streams ht / out.  One 3D instruction batch-transposes
            (hi-lo)*4 independent [128q, 128k] tiles.
            """
            n = (hi - lo) * QT_PER_CHUNK
            out_view = expT_t[:, lo * QT_PER_CHUNK * P : hi * QT_PER_CHUNK * P]
            out3 = out_view.rearrange("p (j q) -> p j q", j=n)
            nc.scalar.dma_start(out3, pn_t[:, lo:hi], transpose=True)

        pdt = mybir.dt.bfloat16 if cfg["pu_bf16"] else mybir.dt.float32
        norm_eng = nc.gpsimd if cfg["norm_pool"] else nc.vector

        def make_dsum():
            dsum = dpool.tile([P, H * 8], pdt)
            rds = dpool.tile([P, H * 8], pdt, tag="rds")
            return dsum, rds

        def emit_scores_pair(c, g, qT_t, pn_t, expT_t, dsum, rds, nsplit=None):
            """scores -> exp -> segmented denominators -> normalized probs
            for head pair g (written into pn_t); kicks the transpose when a
            tsplit boundary is reached."""
            if nsplit is None:
                nsplit = cfg["tsplit"]
            for hh in range(2):
                h = 2 * g + hh
                off = hh * HD
                d0 = h * 8
                sp = sps.tile([P, QT_PER_CHUNK * NK], mybir.dt.float32)
                pu = ppool.tile([P, QT_PER_CHUNK * NK], pdt)
                for qt in range(QT_PER_CHUNK):
                    nc.tensor.matmul(
                        sp[:, qt * NK : (qt + 1) * NK],
                        lhsT=qT_t[off : off + HD, g, qt * P : (qt + 1) * P],
                        rhs=kc_t[off : off + HD, g * NK : (g + 1) * NK],
                        start=True,
                        stop=True,
                    )
                nc.scalar.activation(
                    pu[:],
                    sp[:],
                    mybir.ActivationFunctionType.Exp,
                    scale=float(SCALE),
                )
                pu3 = pu[:].rearrange("p (q k) -> p q k", q=QT_PER_CHUNK)
                with nc.allow_low_precision(reason="93-term softmax denominator; bf16 keeps the DVE reduce in 2x mode"):
                    nc.vector.reduce_sum(
                        dsum[:, d0 : d0 + 4], pu3[:, :, 0:S_TXT],
                        axis=mybir.AxisListType.X,
                    )
                    nc.vector.reduce_sum(
                        dsum[:, d0 + 4 : d0 + 8], pu3[:, :, S_TXT:NK],
                        axis=mybir.AxisListType.X,
                    )
                with nc.allow_low_precision(reason="bf16 reciprocal of softmax denominator"):
                    nc.vector.reciprocal(rds[:, d0 : d0 + 8], dsum[:, d0 : d0 + 8])
                # batched normalize into the staging tile: one op per
                # softmax, broadcasting the per-(partition, qtile)
                # reciprocal along keys (step-0 AP)
                pn3 = pn_t[:, h]  # [128, QT, 128], key cols 93: stay 0
                norm_eng.tensor_tensor(
                    pn3[:, :, 0:S_TXT],
                    pu3[:, :, 0:S_TXT],
                    rds[:, d0 : d0 + 4, None].to_broadcast([P, QT_PER_CHUNK, S_TXT]),
                    mybir.AluOpType.mult,
                )
                norm_eng.tensor_tensor(
                    pn3[:, :, S_TXT:NK],
                    pu3[:, :, S_TXT:NK],
                    rds[:, d0 + 4 : d0 + 8, None].to_broadcast([P, QT_PER_CHUNK, S_AUD]),
                    mybir.AluOpType.mult,
                )
            span = (H // 2) // nsplit
            if (g + 1) % span == 0:
                lo = (g + 1 - span) * 2
                kick_transpose(pn_t, expT_t, lo, (g + 1) * 2)

        def emit_pv_pair(g, expT_t, at_t):
            ev = expT_t[:].rearrange("p (h q) -> p h q", h=H)
            pvp = vps.tile([P, CHUNK], mybir.dt.float32)
            for hh in range(2):
                h = 2 * g + hh
                off = hh * HD
                nc.tensor.matmul(
                    pvp[off : off + HD, :],
                    lhsT=v_t[:, h * HD : (h + 1) * HD],
                    rhs=ev[:, h, :],
                    start=True,
                    stop=True,
                )
            # attn_T tile g holds heads 2g (rows 0-63) and 2g+1 (64-127)
            on_dve = (g % 2 == 1) if cfg["at_alt"] else cfg["at_dve"]
            gate_copy(at_t[:, g, :], pvp[:], cfg["prio_at"], on_dve)

        def emit_outproj_group(c, at_t, j):
            qt, nb = j // 2, j % 2
            op = accps.tile([P, CHUNK], mybir.dt.float32, tag="acc")
            for kt in range(KT):
                nc.tensor.matmul(
                    op[:],
                    lhsT=at_t[:, kt, qt * P : (qt + 1) * P],
                    rhs=wo_t[:, kt * D + nb * CHUNK : kt * D + (nb + 1) * CHUNK],
                    start=(kt == 0),
                    stop=(kt == KT - 1),
                )
            ob = opool.tile([P, CHUNK], mybir.dt.float32)
            on_act = (j % 2 == 0) if cfg["out_alt"] else cfg["out_act"]
            cm = (tc.high_priority(None if cfg["prio_out"] < 0 else cfg["prio_out"])
                  if cfg["prio_out"] else contextlib.nullcontext())
            with cm:
                if on_act:
                    nc.scalar.copy(ob[:], op[:])
                else:
                    nc.vector.tensor_copy(ob[:], op[:])
            nc.sync.dma_start(
                out_d[
                    c * CHUNK + qt * P : c * CHUNK + (qt + 1) * P,
                    nb * CHUNK : (nb + 1) * CHUNK,
                ],
                ob[:],
            )

        chunks = [c for _ in range(reps) for c in range(N_CHUNKS)]
        # prologue: load chunk 0, then qproj-mt g immediately followed by the
        # scores for head pair g (which only needs qT[:, g]) so the ACT/DVE
        # softmax chain overlaps the remaining projections; chunk 1's qproj
        # groups are pulled in as extra PE filler under the same chain.
        ht_t = emit_ht(chunks[0], ht_t0)
        qT_pre = None
        if len(chunks) > 1:
            ht_next = emit_ht(chunks[1])
            qT_pre = qpool.tile([P, KT, CHUNK], mybir.dt.bfloat16)
        qT_t = qpool.tile([P, KT, CHUNK], mybir.dt.bfloat16)
        dsum, rds = make_dsum()
        for g in range(KT):
            emit_qproj_mt(ht_t, qT_t, g)
            emit_scores_pair(chunks[0], g, qT_t, pn_bufs[0], expT_bufs[0],
                             dsum, rds, nsplit=cfg["tsplit0"])
            if qT_pre is not None and cfg["deep_prologue"]:
                emit_qproj_mt(ht_next, qT_pre, g)
        for i, c in enumerate(chunks):
            nxt = chunks[i + 1] if i + 1 < len(chunks) else None
            if i + 2 < len(chunks):
                ht_fut = emit_ht(chunks[i + 2])
            # phase 1: qproj(c+1) interleaved with pv(c)
            at_t = atpool.tile([P, KT, CHUNK], mybir.dt.bfloat16, tag="at")
            pre_done = i == 0 and cfg["deep_prologue"] and qT_pre is not None
            if nxt is not None:
                qT_next = (qT_pre if pre_done
                           else qpool.tile([P, KT, CHUNK], mybir.dt.bfloat16))
            if cfg["phase1_split"]:
                if nxt is not None and not pre_done:
                    for g in range(KT):
                        emit_qproj_mt(ht_next, qT_next, g)
                for g in range(KT):
                    emit_pv_pair(g, expT_bufs[i % 2], at_t)
            else:
                for g in range(KT):
                    if nxt is not None and not pre_done:
                        emit_qproj_mt(ht_next, qT_next, g)
                    emit_pv_pair(g, expT_bufs[i % 2], at_t)
            # phase 2: scores/norm(c+1) interleaved with outproj(c)
            if nxt is not None:
                dsum, rds = make_dsum()
            for g in range(KT):
                if nxt is not None:
                    emit_scores_pair(nxt, g, qT_next, pn_bufs[(i + 1) % 2],
                                     expT_bufs[(i + 1) % 2], dsum, rds)
                emit_outproj_group(c, at_t, g)
            if nxt is not None:
                qT_t = qT_next
                ht_next = ht_fut if i + 2 < len(chunks) else None

    nc.compile()
    return nc


def _host_prep(hidden_states, encoder_hidden_states, audio_hidden_states,
               Wq, Wk, Wv, Wk_audio, Wv_audio, Wo):
    """Build the per-core input maps (all layouts pre-arranged on host)."""
    # mt-major: wq_sb[p, (mt, kt, c)] = Wq[kt*128+p, mt*128+c] so each qproj
    # output-column group's weights are one contiguous 256KB DMA slice
    wq_sb = np.ascontiguousarray(
        Wq.reshape(KT, P, KT, P).transpose(1, 2, 0, 3).reshape(P, KT * D)
    ).astype(BF16)
    wo_sb = np.ascontiguousarray(
        Wo.reshape(KT, P, D).transpose(1, 0, 2).reshape(P, KT * D)
    ).astype(BF16)

    in_maps = []
    for b in range(B):
        # kv projections for this batch: tiny, done on host
        k_full = np.concatenate(
            [encoder_hidden_states[b] @ Wk, audio_hidden_states[b] @ Wk_audio], axis=0
        )  # [93, 1024]
        v_full = np.concatenate(
            [encoder_hidden_states[b] @ Wv, audio_hidden_states[b] @ Wv_audio], axis=0
        )  # [93, 1024]
        kc_sb = np.ascontiguousarray(
            k_full.T.reshape(KT, P, NK).transpose(1, 0, 2).reshape(P, KT * NK)
        ).astype(BF16)
        v_sb = np.zeros((P, D), dtype=BF16)
        v_sb[:NK] = v_full.astype(BF16)

        for half in range(2):
            rows = hidden_states[b, half * SEQ_PER_CORE : (half + 1) * SEQ_PER_CORE]
            ht_sb = np.ascontiguousarray(
                rows.T.reshape(KT, P, SEQ_PER_CORE)
                .transpose(1, 0, 2)
                .reshape(P, KT * SEQ_PER_CORE)
            ).astype(BF16)
            in_maps.append(
                {"ht": ht_sb, "wq": wq_sb, "wo": wo_sb, "kc": kc_sb, "v": v_sb}
            )
    return in_maps


_NC_CACHE = {}


def get_nc():
    if "nc" not in _NC_CACHE:
        _NC_CACHE["nc"] = build_bass()
    return _NC_CACHE["nc"]


def kernel(hidden_states, encoder_hidden_states, audio_hidden_states,
           Wq, Wk, Wv, Wk_audio, Wv_audio, Wo, bo):
    from concourse import bass_utils

    hidden_states = np.asarray(hidden_states, dtype=np.float32)
    encoder_hidden_states = np.asarray(encoder_hidden_states, dtype=np.float32)
    audio_hidden_states = np.asarray(audio_hidden_states, dtype=np.float32)
    Wq = np.asarray(Wq, dtype=np.float32)
    Wk = np.asarray(Wk, dtype=np.float32)
    Wv = np.asarray(Wv, dtype=np.float32)
    Wk_audio = np.asarray(Wk_audio, dtype=np.float32)
    Wv_audio = np.asarray(Wv_audio, dtype=np.float32)
    Wo = np.asarray(Wo, dtype=np.float32)
    bo = np.asarray(bo, dtype=np.float32)

    nc = get_nc()
    in_maps = _host_prep(hidden_states, encoder_hidden_states, audio_hidden_states,
                         Wq, Wk, Wv, Wk_audio, Wv_audio, Wo)
    res = bass_utils.run_bass_kernel_spmd(nc, in_maps, list(range(N_CORES)))

    out = np.empty((B, S, D), dtype=np.float32)
    core = 0
    for b in range(B):
        for half in range(2):
            out[b, half * SEQ_PER_CORE : (half + 1) * SEQ_PER_CORE] = res.results[core]["out"]
            core += 1
    out += bo[None, None, :]
    return out
